# revision 16
# baseline (speedup 1.0000x reference)
"""Trainium2 Bass kernel for a single transformer encoder layer.

Problem: src [8, 1024, 512], 8-head self-attention (d=512, hd=64),
FFN 512->128->512, two post-residual LayerNorms (torch encoder-layer,
norm_first=False), eval mode.

Sharding: data-parallel over batch -- each of the 8 NeuronCores gets one
batch element [1024, 512] and runs the full layer on it.

Layout strategy (per core):
  - All matmul contractions put the contracted dim on SBUF partitions.
  - Host pre-transposes src (srcT [512,1024]) and all weights so both
    matmul operands are contiguous loads.
  - Q,K are produced transposed (channels on partitions) for the scores
    matmul; V is produced natural [s, c] padded with a ones column so the
    attn@V matmul also emits the softmax denominator row for free.
  - softmax skips max-subtraction: scores = q.k/8 with q,k ~ N(0, 1/3)
    are bounded by ~+-3, exp() is far from overflow in fp32.
  - LayerNorm gamma/beta of LN1 are folded into the FFN weights on the
    host (FFN consumes the pre-affine normalized xhat).
"""

import sys

for _p in ("/opt/trn_rl_repo",):
    if _p not in sys.path:
        sys.path.insert(0, _p)

import numpy as np

import concourse.bass as bass
import concourse.mybir as mybir
import concourse.tile as tile
from concourse import bacc
from concourse.bass_utils import run_bass_kernel_spmd
from concourse.masks import make_identity

F32 = mybir.dt.float32
ALU = mybir.AluOpType
ACTF = mybir.ActivationFunctionType

B = 8          # batch == number of cores
S = 1024       # sequence length
D = 512        # model dim
H = 8          # heads
HD = 64        # head dim
FF = 128       # ffn dim
EPS = 1e-5
P = 128        # partitions
SC = S // P    # 8 s-chunks
DC = D // P    # 4 d-chunks
QKC = (2 * D) // P  # 8 qk channel chunks
SB = S // 512  # 2 s-blocks of 512

_CACHED = {}


def build_bass():
    nc = bacc.Bacc(None, target_bir_lowering=False)

    # ---- DRAM I/O ----------------------------------------------------
    a_srcT = nc.declare_dram_parameter("srcT", [D, S], F32, False)
    a_src = nc.declare_dram_parameter("src", [S, D], F32, False)
    a_winT = nc.declare_dram_parameter("winT", [D, 3 * D], F32, False)
    a_woT = nc.declare_dram_parameter("woT", [D, D], F32, False)
    a_w1T = nc.declare_dram_parameter("w1Tp", [D, FF], F32, False)
    a_w2T = nc.declare_dram_parameter("w2T", [FF, D], F32, False)
    a_inb = nc.declare_dram_parameter("inb", [3 * D], F32, False)
    a_outb = nc.declare_dram_parameter("outb", [D], F32, False)
    a_b1p = nc.declare_dram_parameter("b1p", [FF], F32, False)
    a_b2 = nc.declare_dram_parameter("b2", [D], F32, False)
    a_g1 = nc.declare_dram_parameter("g1", [D], F32, False)
    a_be1 = nc.declare_dram_parameter("be1", [D], F32, False)
    a_g2 = nc.declare_dram_parameter("g2", [D], F32, False)
    a_be2 = nc.declare_dram_parameter("be2", [D], F32, False)
    a_out = nc.declare_dram_parameter("out", [S, D], F32, True)

    def bcast(vec, n):
        # DRAM vector [n] -> AP replicated across 128 partitions
        vec_ap = vec[:]
        return bass.AP(
            tensor=vec_ap.tensor, offset=vec_ap.offset, ap=[[0, P], [1, n]]
        )


    def dve_rsqrt(nc, out_ap, var_ap, tmp_pool, n):
        """out = 1/sqrt(var + EPS) via bit-trick seed + 3 Newton steps (DVE only,
        avoids ACT sqrt-table switches)."""
        ti = tmp_pool.tile([P, n], mybir.dt.int32, tag="rsq_i", name="rsq_i")
        tv = tmp_pool.tile([P, n], F32, tag="rsq_v", name="rsq_v")
        ty = tmp_pool.tile([P, n], F32, tag="rsq_y", name="rsq_y")
        tt = tmp_pool.tile([P, n], F32, tag="rsq_t", name="rsq_t")
        nc.vector.tensor_scalar_add(tv[:], var_ap, EPS)
        # seed: y0 = bitcast(0x5f3759df - (bitcast_i32(v) >> 1))
        nc.vector.tensor_scalar(
            out=ti[:], in0=tv[:].bitcast(mybir.dt.int32), scalar1=1, scalar2=None,
            op0=ALU.logical_shift_right,
        )
        nc.vector.tensor_scalar(
            out=ti[:], in0=ti[:], scalar1=0x5F3759DF, scalar2=-1,
            op0=ALU.subtract, op1=ALU.mult,
        )
        nc.vector.tensor_copy(out=ty[:], in_=ti[:].bitcast(F32))
        for _ in range(3):
            nc.vector.tensor_tensor(out=tt[:], in0=ty[:], in1=ty[:], op=ALU.mult)
            nc.vector.tensor_tensor(out=tt[:], in0=tt[:], in1=tv[:], op=ALU.mult)
            nc.vector.tensor_scalar(
                out=tt[:], in0=tt[:], scalar1=-0.5, scalar2=1.5,
                op0=ALU.mult, op1=ALU.add,
            )
            nc.vector.tensor_tensor(out=ty[:], in0=ty[:], in1=tt[:], op=ALU.mult)
        nc.vector.tensor_copy(out=out_ap, in_=ty[:])

    with tile.TileContext(nc) as tc:
        with (
            tc.tile_pool(name="persist", bufs=1) as persist,
            tc.tile_pool(name="small", bufs=1) as small,
        ):
            # ---- persistent weights / constants ----------------------
            t_src = persist.tile([P, SC, D], F32, tag="src")      # [s] natural, becomes y then x
            t_woT = persist.tile([P, DC, D], F32, tag="woT")
            t_w1T = persist.tile([P, DC, FF], F32, tag="w1T")
            t_w2T = persist.tile([P, D], F32, tag="w2T")
            t_ctxT = persist.tile([P, DC, S], F32, tag="ctxT")
            t_xhat = persist.tile([P, SC, D], F32, tag="xhat")
            t_xhatT = persist.tile([P, DC, S], F32, tag="xhatT")
            t_h1T = persist.tile([P, S], F32, tag="h1T")

            t_g1bc = persist.tile([P, D], F32, tag="g1bc")
            t_be1bc = persist.tile([P, D], F32, tag="be1bc")
            t_g2bc = persist.tile([P, D], F32, tag="g2bc")
            t_be2bc = persist.tile([P, D], F32, tag="be2bc")

            t_inb = small.tile([1, 3 * D], F32, tag="inb")
            t_outb = small.tile([1, D], F32, tag="outb")
            t_b2 = small.tile([1, D], F32, tag="b2")
            t_b1p = small.tile([FF, 1], F32, tag="b1p")
            t_ones512 = small.tile([1, 512], F32, tag="ones512")
            t_ones128 = small.tile([1, P], F32, tag="ones128")
            t_ones64 = small.tile([1, HD], F32, tag="ones64")
            t_eps = small.tile([P, 1], F32, tag="eps")
            t_ident = small.tile([P, P], F32, tag="ident")

            # stats scratch [128, SC]
            t_sum1 = small.tile([P, SC], F32, tag="sum1")
            t_sq1 = small.tile([P, SC], F32, tag="sq1")
            t_mu1 = small.tile([P, SC], F32, tag="mu1")
            t_var1 = small.tile([P, SC], F32, tag="var1")
            t_rsig1 = small.tile([P, SC], F32, tag="rsig1")
            t_bp1 = small.tile([P, SC], F32, tag="bp1")
            t_sum2 = small.tile([P, SC], F32, tag="sum2")
            t_sq2 = small.tile([P, SC], F32, tag="sq2")
            t_mu2 = small.tile([P, SC], F32, tag="mu2")
            t_var2 = small.tile([P, SC], F32, tag="var2")
            t_rsig2 = small.tile([P, SC], F32, tag="rsig2")
            t_bp2 = small.tile([P, SC], F32, tag="bp2")

            # ---- phase 0: loads --------------------------------------
            for sc in range(SC):
                nc.sync.dma_start(out=t_src[:, sc, :], in_=a_src[sc * P:(sc + 1) * P, :])
            for dc in range(DC):
                nc.sync.dma_start(out=t_woT[:, dc, :], in_=a_woT[dc * P:(dc + 1) * P, :])
                nc.sync.dma_start(out=t_w1T[:, dc, :], in_=a_w1T[dc * P:(dc + 1) * P, :])
            nc.sync.dma_start(out=t_w2T[:], in_=a_w2T[:, :])
            nc.gpsimd.dma_start(out=t_g1bc[:], in_=bcast(a_g1, D))
            nc.gpsimd.dma_start(out=t_be1bc[:], in_=bcast(a_be1, D))
            nc.gpsimd.dma_start(out=t_g2bc[:], in_=bcast(a_g2, D))
            nc.gpsimd.dma_start(out=t_be2bc[:], in_=bcast(a_be2, D))
            nc.sync.dma_start(out=t_inb[:], in_=a_inb[None, :])
            nc.sync.dma_start(out=t_outb[:], in_=a_outb[None, :])
            nc.sync.dma_start(out=t_b2[:], in_=a_b2[None, :])
            nc.sync.dma_start(out=t_b1p[:], in_=a_b1p[:, None])
            nc.vector.memset(t_ones512[:], 1.0)
            nc.vector.memset(t_ones128[:], 1.0)
            nc.vector.memset(t_ones64[:], 1.0)
            nc.vector.memset(t_eps[:], EPS)
            make_identity(nc, t_ident[:])

            with tc.tile_pool(name="qkbuf", bufs=1) as qkbuf:
                # qkT: q,k channels on partitions  [8 chunks][128, 1024]
                t_qkT = qkbuf.tile([P, QKC, S], F32, tag="qkT")
                # v natural padded: [s-chunk][128, 8 heads, 65]
                t_vaug = qkbuf.tile([P, SC, H, HD + 1], F32, tag="vaug")

                # ================= phase 1: QKV projections ==========
                with (
                    tc.tile_pool(name="ld1", bufs=1) as ld1,
                    tc.tile_pool(name="ps1", bufs=4, space="PSUM") as ps1,
                ):
                    t_srcT = ld1.tile([P, DC, S], F32, tag="srcT")
                    t_winT = ld1.tile([P, DC, 3 * D], F32, tag="winT")
                    for dc in range(DC):
                        nc.sync.dma_start(
                            out=t_srcT[:, dc, :], in_=a_srcT[dc * P:(dc + 1) * P, :]
                        )
                        nc.sync.dma_start(
                            out=t_winT[:, dc, :], in_=a_winT[dc * P:(dc + 1) * P, :]
                        )

                    # qkT[c, s] = sum_d winT[d, c] * srcT[d, s] + inb[c]
                    for cc in range(QKC):
                        for sb in range(SB):
                            ps = ps1.tile([P, 512], F32, tag="mm")
                            for dc in range(DC):
                                nc.tensor.matmul(
                                    ps[:],
                                    lhsT=t_winT[:, dc, cc * P:(cc + 1) * P],
                                    rhs=t_srcT[:, dc, sb * 512:(sb + 1) * 512],
                                    start=(dc == 0),
                                    stop=False,
                                )
                            nc.tensor.matmul(
                                ps[:],
                                lhsT=t_inb[:, cc * P:(cc + 1) * P],
                                rhs=t_ones512[:],
                                start=False,
                                stop=True,
                            )
                            nc.vector.tensor_copy(
                                out=t_qkT[:, cc, sb * 512:(sb + 1) * 512], in_=ps[:]
                            )

                    # v natural [s, c] (+bias) into padded vaug
                    for sc in range(SC):
                        ps = ps1.tile([P, 512], F32, tag="mm")
                        for dc in range(DC):
                            nc.tensor.matmul(
                                ps[:],
                                lhsT=t_srcT[:, dc, sc * P:(sc + 1) * P],
                                rhs=t_winT[:, dc, 2 * D:3 * D],
                                start=(dc == 0),
                                stop=False,
                            )
                        nc.tensor.matmul(
                            ps[:],
                            lhsT=t_ones128[:],
                            rhs=t_inb[:, 2 * D:3 * D],
                            start=False,
                            stop=True,
                        )
                        nc.vector.tensor_copy(
                            out=t_vaug[:, sc, :, 0:HD],
                            in_=ps[:].rearrange("p (h d) -> p h d", h=H),
                        )
                        nc.vector.memset(t_vaug[:, sc, :, HD:HD + 1], 1.0)

                # ================= phase 2: attention ================
                with (
                    tc.tile_pool(name="pssc", bufs=2, space="PSUM") as pssc,
                    tc.tile_pool(name="psctx", bufs=2, space="PSUM") as psctx,
                    tc.tile_pool(name="psrb", bufs=2, space="PSUM") as psrb,
                    tc.tile_pool(name="expbuf", bufs=3) as expbuf,
                    tc.tile_pool(name="attnsm", bufs=2) as attnsm,
                ):
                    for h in range(H):
                        qc = h // 2          # q chunk index in qkT
                        kc = 4 + h // 2      # k chunk index in qkT
                        po = (h % 2) * HD    # partition offset within chunk
                        ctx_ps = []
                        for _sb in range(SB):
                            cps = psctx.tile([HD + 1, 512], F32, tag="ctx", name=f"ctx_{h}_{_sb}")
                            ctx_ps.append(cps)
                        for sk in range(SC):
                            sps = pssc.tile([P, S], F32, tag="scores")
                            for sb in range(SB):
                                nc.tensor.matmul(
                                    sps[:, sb * 512:(sb + 1) * 512],
                                    lhsT=t_qkT[po:po + HD, kc, sk * P:(sk + 1) * P],
                                    rhs=t_qkT[po:po + HD, qc, sb * 512:(sb + 1) * 512],
                                    start=True,
                                    stop=True,
                                )
                            texp = expbuf.tile([P, S], F32, tag="expT")
                            nc.scalar.activation(
                                out=texp[:], in_=sps[:], func=ACTF.Exp,
                                bias=0.0, scale=0.125,
                            )
                            for sb in range(SB):
                                nc.tensor.matmul(
                                    ctx_ps[sb][:],
                                    lhsT=t_vaug[:, sk, h, :],
                                    rhs=texp[:, sb * 512:(sb + 1) * 512],
                                    start=(sk == 0),
                                    stop=(sk == SC - 1),
                                )
                        # normalize: ctxT[c, s] = ctx_ps[0:64] / den(row 64)
                        for sb in range(SB):
                            rden = attnsm.tile([1, 512], F32, tag="rden")
                            nc.vector.reciprocal(
                                out=rden[:], in_=ctx_ps[sb][HD:HD + 1, :]
                            )
                            rb = psrb.tile([HD, 512], F32, tag="rb")
                            nc.tensor.matmul(
                                rb[:], lhsT=t_ones64[:], rhs=rden[:],
                                start=True, stop=True,
                            )
                            csb = attnsm.tile([HD, 512], F32, tag="csb")
                            nc.vector.tensor_copy(out=csb[:], in_=ctx_ps[sb][0:HD, :])
                            nc.vector.tensor_tensor(
                                out=t_ctxT[po:po + HD, qc, sb * 512:(sb + 1) * 512],
                                in0=csb[:],
                                in1=rb[:],
                                op=ALU.mult,
                            )

            tc.strict_bb_all_engine_barrier()
            # ================= phase 3: out-proj + residual + LN1 ====
            with (
                tc.tile_pool(name="ps3", bufs=3, space="PSUM") as ps3,
                tc.tile_pool(name="sqb3", bufs=2) as sqb3,
            ):
                for g in range(2):
                  gsl = slice(4 * g, 4 * g + 4)
                  for sc in range(4 * g, 4 * g + 4):
                    ps = ps3.tile([P, D], F32, tag="mm")
                    for dc in range(DC):
                        nc.tensor.matmul(
                            ps[:],
                            lhsT=t_ctxT[dc][:, sc * P:(sc + 1) * P],
                            rhs=t_woT[dc][:],
                            start=(dc == 0),
                            stop=False,
                        )
                    nc.tensor.matmul(
                        ps[:], lhsT=t_ones128[:], rhs=t_outb[:],
                        start=False, stop=True,
                    )
                    # y = attn_out + src -> xhat tile (no aliasing)
                    nc.vector.tensor_tensor(
                        out=t_xhat[sc][:], in0=ps[:], in1=t_src[sc][:], op=ALU.add
                    )
                    sq = sqb3.tile([P, D], F32, tag="sq")
                    nc.scalar.activation(
                        out=sq[:], in_=t_xhat[sc][:], func=ACTF.Identity,
                        accum_out=t_sum1[:, sc:sc + 1],
                    )
                    # sumsq via ACT Square with accumulate
                    sq2 = sqb3.tile([P, D], F32, tag="sq2")
                    nc.scalar.activation(
                        out=sq2[:], in_=t_xhat[sc][:], func=ACTF.Square,
                        accum_out=t_sq1[:, sc:sc + 1],
                    )
                  # batched LN1 stats (per 4-chunk group)
                  nc.vector.tensor_scalar_mul(t_mu1[:, gsl], t_sum1[:, gsl], 1.0 / D)
                  nc.vector.tensor_scalar_mul(t_var1[:, gsl], t_sq1[:, gsl], 1.0 / D)
                  nc.vector.tensor_tensor(
                      out=t_bp1[:, gsl], in0=t_mu1[:, gsl], in1=t_mu1[:, gsl], op=ALU.mult
                  )
                  nc.vector.tensor_sub(t_var1[:, gsl], t_var1[:, gsl], t_bp1[:, gsl])
                  dve_rsqrt(nc, t_rsig1[:, gsl], t_var1[:, gsl], sqb3, 4)
                  nc.vector.tensor_tensor(
                      out=t_bp1[:, gsl], in0=t_mu1[:, gsl], in1=t_rsig1[:, gsl], op=ALU.mult
                  )
                  nc.vector.tensor_scalar_mul(t_bp1[:, gsl], t_bp1[:, gsl], -1.0)
                  # apply: xhat = y*rsig + bp ; x = xhat*g1 + be1 (over src)
                  for sc in range(4 * g, 4 * g + 4):
                    nc.scalar.activation(
                        out=t_xhat[sc][:], in_=t_xhat[sc][:],
                        func=ACTF.Identity,
                        bias=t_bp1[:, sc:sc + 1], scale=t_rsig1[:, sc:sc + 1],
                    )
                    nc.vector.tensor_tensor(
                        out=t_src[sc][:], in0=t_xhat[sc][:], in1=t_g1bc[:],
                        op=ALU.mult,
                    )
                    nc.gpsimd.tensor_add(t_src[sc][:], t_src[sc][:], t_be1bc[:])
            # ================= phase 4: transpose xhat ===============
            with tc.tile_pool(name="pstp", bufs=4, space="PSUM") as pstp:
                for sc in range(SC):
                    for dc in range(DC):
                        tp = pstp.tile([P, P], F32, tag="tp")
                        nc.tensor.transpose(
                            tp[:], t_xhat[:, sc, dc * P:(dc + 1) * P], t_ident[:]
                        )
                        nc.vector.tensor_copy(
                            out=t_xhatT[:, dc, sc * P:(sc + 1) * P], in_=tp[:]
                        )

            tc.strict_bb_all_engine_barrier()
            # ================= phase 5: FFN + residual + LN2 =========
            with (
                tc.tile_pool(name="ps5", bufs=2, space="PSUM") as ps5,
                tc.tile_pool(name="sqb5", bufs=2) as sqb5,
                tc.tile_pool(name="obuf", bufs=3) as obuf,
            ):
                # h1T[f, s] = relu(sum_e w1T[e, f] * xhatT[e, s] + b1p)
                for sb in range(SB):
                    ps = ps5.tile([FF, 512], F32, tag="h1")
                    for dc in range(DC):
                        nc.tensor.matmul(
                            ps[:],
                            lhsT=t_w1T[:, dc, :],
                            rhs=t_xhatT[:, dc, sb * 512:(sb + 1) * 512],
                            start=(dc == 0),
                            stop=(dc == DC - 1),
                        )
                    nc.scalar.activation(
                        out=t_h1T[:, sb * 512:(sb + 1) * 512], in_=ps[:],
                        func=ACTF.Relu, bias=t_b1p[:], scale=1.0,
                    )
                # ff[s, e] + x residual + LN2 (two 4-chunk groups)
                for g in range(2):
                  gsl = slice(4 * g, 4 * g + 4)
                  for sc in range(4 * g, 4 * g + 4):
                    ps = ps5.tile([P, D], F32, tag="mm")
                    nc.tensor.matmul(
                        ps[:],
                        lhsT=t_h1T[:, sc * P:(sc + 1) * P],
                        rhs=t_w2T[:],
                        start=True,
                        stop=False,
                    )
                    nc.tensor.matmul(
                        ps[:], lhsT=t_ones128[:], rhs=t_b2[:],
                        start=False, stop=True,
                    )
                    # y2 = ff + x -> xhat tile (xhat free after transpose)
                    nc.vector.tensor_tensor(
                        out=t_xhat[sc][:], in0=ps[:], in1=t_src[sc][:], op=ALU.add
                    )
                    sq = sqb5.tile([P, D], F32, tag="sq")
                    nc.scalar.activation(
                        out=sq[:], in_=t_xhat[sc][:], func=ACTF.Identity,
                        accum_out=t_sum2[:, sc:sc + 1],
                    )
                    sq2 = sqb5.tile([P, D], F32, tag="sq2")
                    nc.scalar.activation(
                        out=sq2[:], in_=t_xhat[sc][:], func=ACTF.Square,
                        accum_out=t_sq2[:, sc:sc + 1],
                    )
                  nc.vector.tensor_scalar_mul(t_mu2[:, gsl], t_sum2[:, gsl], 1.0 / D)
                  nc.vector.tensor_scalar_mul(t_var2[:, gsl], t_sq2[:, gsl], 1.0 / D)
                  nc.vector.tensor_tensor(
                      out=t_bp2[:, gsl], in0=t_mu2[:, gsl], in1=t_mu2[:, gsl], op=ALU.mult
                  )
                  nc.vector.tensor_sub(t_var2[:, gsl], t_var2[:, gsl], t_bp2[:, gsl])
                  dve_rsqrt(nc, t_rsig2[:, gsl], t_var2[:, gsl], sqb5, 4)
                  nc.vector.tensor_tensor(
                      out=t_bp2[:, gsl], in0=t_mu2[:, gsl], in1=t_rsig2[:, gsl], op=ALU.mult
                  )
                  nc.vector.tensor_scalar_mul(t_bp2[:, gsl], t_bp2[:, gsl], -1.0)
                  for sc in range(4 * g, 4 * g + 4):
                    ot = obuf.tile([P, D], F32, tag="ot")
                    nc.scalar.activation(
                        out=ot[:], in_=t_xhat[sc][:], func=ACTF.Identity,
                        bias=t_bp2[:, sc:sc + 1], scale=t_rsig2[:, sc:sc + 1],
                    )
                    nc.vector.tensor_tensor(
                        out=ot[:], in0=ot[:], in1=t_g2bc[:], op=ALU.mult
                    )
                    nc.gpsimd.tensor_add(ot[:], ot[:], t_be2bc[:])
                    nc.sync.dma_start(out=a_out[sc * P:(sc + 1) * P, :], in_=ot[:])

    nc.finalize()
    return nc


def _prep_in_maps(inputs):
    src = np.ascontiguousarray(np.asarray(inputs["src"], dtype=np.float32))
    in_proj_w = np.asarray(inputs["in_proj_w"], dtype=np.float32)
    in_proj_b = np.asarray(inputs["in_proj_b"], dtype=np.float32)
    out_proj_w = np.asarray(inputs["out_proj_w"], dtype=np.float32)
    out_proj_b = np.asarray(inputs["out_proj_b"], dtype=np.float32)
    w1 = np.asarray(inputs["w1"], dtype=np.float32)
    b1 = np.asarray(inputs["b1"], dtype=np.float32)
    w2 = np.asarray(inputs["w2"], dtype=np.float32)
    b2 = np.asarray(inputs["b2"], dtype=np.float32)
    g1 = np.asarray(inputs["g1"], dtype=np.float32)
    be1 = np.asarray(inputs["be1"], dtype=np.float32)
    g2 = np.asarray(inputs["g2"], dtype=np.float32)
    be2 = np.asarray(inputs["be2"], dtype=np.float32)

    winT = np.ascontiguousarray(in_proj_w.T)          # [D, 3D]
    woT = np.ascontiguousarray(out_proj_w.T)          # [D, D]
    # fold LN1 affine into FFN first layer
    w1Tp = np.ascontiguousarray((w1 * g1[None, :]).T)  # [D, FF]
    b1p = (b1 + w1 @ be1).astype(np.float32)           # [FF]

    shared = dict(
        winT=winT, woT=woT, w1Tp=w1Tp, w2T=np.ascontiguousarray(w2.T),
        inb=in_proj_b, outb=out_proj_b, b1p=b1p, b2=b2,
        g1=g1, be1=be1, g2=g2, be2=be2,
        ones=np.ones((512,), np.float32),
    )
    in_maps = []
    for i in range(B):
        m = dict(shared)
        m["src"] = np.ascontiguousarray(src[i])
        m["srcT"] = np.ascontiguousarray(src[i].T)
        in_maps.append(m)
    return in_maps


def _run(inputs, trace=False):
    if "nc" not in _CACHED:
        _CACHED["nc"] = build_bass()
    nc = _CACHED["nc"]
    in_maps = _prep_in_maps(inputs)
    res = run_bass_kernel_spmd(nc, in_maps, list(range(B)), trace=trace)
    out = np.stack([np.asarray(res.results[i]["out"]) for i in range(B)])
    return out.astype(np.float32), res


def kernel(**inputs):
    out, _ = _run(inputs, trace=False)
    return out
